# revision 71
# baseline (speedup 1.0000x reference)
"""Trainium2 Bass kernel for nn_ContinuousGenHyperConnections.

Math: per token t (row x of length 2048, viewed as 4 streams of 512):
    s    = 1/sqrt(mean(x^2) + eps)                  (RMSNorm scale)
    F    = (x @ Wall^T) * s + C                      (42 tiny projections, fused)
    wr   = sigmoid(F[32:36]); dt = eps_affine(sigmoid(F[36:38])); ww = F[38:42]
    A    = dt_c*(M - M^T) - (dt_d/2) * (R R^T),  M = F[0:16], R = F[16:32]
    u    = wr + wr @ A;  D = A + ww (x) u            (G = I + D collapses the
                                                      whole module: out = G h)
    out  = x + sum_j D[i,j] * x[stream j]            (per-stream mixing)

Kernel layout: tokens on partitions (128/tile). Projections via PE matmul
(needs per-tile PE transposes of x to put d on partitions). Stream mixing on
PE as diagonal matmuls (diag(D_ij) as stationary) accumulating in PSUM; the
identity term is added as fp32 on DVE from the original x tile.

Sharding: pure data parallel over B*T across 8 cores, params replicated.
"""

import numpy as np
import ml_dtypes

import concourse.bacc as bacc
import concourse.tile as tile
from concourse import mybir
from concourse.bass_utils import run_bass_kernel_spmd

F32 = mybir.dt.float32
BF16 = mybir.dt.bfloat16
AF = mybir.ActivationFunctionType
OP = mybir.AluOpType

D = 2048
NSTR = 4
BS = 512
NF = 48
P = 128
NCORES = 8
NBLK = D // P  # 16 d-blocks
EPS = float(np.finfo(np.float32).eps)
DT_MIN, DT_MAX = 1e-3, 1.0

TRACE = False
LAST_RESULTS = None  # BassKernelResults of the last run (for test harness)

_NC_CACHE = {}


def _load_act_set(nc, name="natural_log_exp_and_others"):
    """Preload the one ACT table set covering Square/Ln/Exp so bacc's greedy
    per-function chooser doesn't alternate sets (~2.7us per reload)."""
    from concourse.hw_specs import get_activation_tables
    tables = get_activation_tables(nc.m.arch)
    set_id = list(tables).index(name)
    li = mybir.InstLoadActFuncSet(
        name=nc.get_next_instruction_name(), ins=[], outs=[],
        act_func_set_id=set_id)
    return nc.scalar.add_instruction(li)


def build_nc(tpc):
    """Build the Bass module for one core processing `tpc` tokens."""
    assert tpc % P == 0
    nt = tpc // P
    nc = bacc.Bacc("TRN2", target_bir_lowering=False)

    x_in = nc.dram_tensor("x", [tpc, D], F32, kind="ExternalInput")
    wt_in = nc.dram_tensor("wt", [P, NBLK * NF], BF16, kind="ExternalInput")
    cv_in = nc.dram_tensor("cvec", [P, NF], F32, kind="ExternalInput")
    id_in = nc.dram_tensor("ident", [P, P], BF16, kind="ExternalInput")
    out_d = nc.dram_tensor("out", [tpc, D], F32, kind="ExternalOutput")

    with tile.TileContext(nc) as tc:
        with (
            tc.tile_pool(name="consts", bufs=1) as consts,
            tc.tile_pool(name="xp", bufs=5) as xp,
            tc.tile_pool(name="xhp", bufs=3) as xhp,
            tc.tile_pool(name="xtp", bufs=3) as xtp,
            tc.tile_pool(name="op_", bufs=5) as op_,
            tc.tile_pool(name="sqp", bufs=1) as sqp,
            tc.tile_pool(name="dgp", bufs=3) as dgp,
            tc.tile_pool(name="mxs", bufs=4) as mxs,
            tc.tile_pool(name="small", bufs=4) as small,
            tc.tile_pool(name="tp_ps", bufs=3, space="PSUM") as tp_ps,
            tc.tile_pool(name="pj_ps", bufs=1, space="PSUM") as pj_ps,
            tc.tile_pool(name="mx_ps", bufs=4, space="PSUM") as mx_ps,
        ):
            wt_s = consts.tile([P, NBLK, NF], BF16)
            nc.sync.dma_start(out=wt_s, in_=wt_in.ap().rearrange("p (k f) -> p k f", k=NBLK))
            cv_s = consts.tile([P, NF], F32)
            nc.sync.dma_start(out=cv_s, in_=cv_in.ap())
            id_s = consts.tile([P, P], BF16)
            nc.sync.dma_start(out=id_s, in_=id_in.ap())
            _load_act_set(nc)

            sq = sqp.tile([P, D], F32)  # dummy target for square pass

            # software prefetch: keep 3 loads ahead of compute so a store's
            # sem wait on the SP queue never starves the next tile's load
            PF = 3
            x_tiles = {}

            def _load(t):
                if t < nt:
                    xt = xp.tile([P, D], F32, name="x_t")
                    nc.sync.dma_start(out=xt, in_=x_in[t * P:(t + 1) * P, :])
                    x_tiles[t] = xt

            for t in range(PF):
                _load(t)

            for t in range(nt):
                x_t = x_tiles.pop(t)

                # --- RMS scale: s = exp(-0.5 * ln(mean(x^2))) ---
                # (ln/exp keep ACT on ONE table set together with the
                # exp-based sigmoid below; Sqrt/Sigmoid would thrash
                # ~2.7us table loads every tile. eps ~1.2e-7 negligible.)
                ssq = small.tile([P, 1], F32)
                nc.scalar.activation(out=sq, in_=x_t, func=AF.Square, accum_out=ssq)
                lm = small.tile([P, 1], F32)
                nc.scalar.activation(out=lm, in_=ssq, func=AF.Ln, scale=1.0 / D)
                s = small.tile([P, 1], F32)
                nc.scalar.activation(out=s, in_=lm, func=AF.Exp, scale=-0.5)

                # --- cast to bf16 for PE ---
                xh = xhp.tile([P, D], BF16)
                nc.vector.tensor_copy(out=xh, in_=x_t)

                # --- transposes: xh [tok, d] -> xt [d, tok] in 16 blocks ---
                # (regular matmul vs identity: transpose-MMs only get one HW
                # wait slot and walrus rejects the 2-wait schedule Tile emits)
                xt_t = xtp.tile([P, NBLK, P], BF16)
                for g in range(2):
                    tp = tp_ps.tile([P, 8, P], BF16, tag="tp")
                    for b in range(8):
                        k = 8 * g + b
                        nc.tensor.transpose(tp[:, b, :], xh[:, k * P:(k + 1) * P],
                                            id_s)
                    nc.scalar.copy(out=xt_t[:, 8 * g:8 * g + 8, :], in_=tp)

                # --- projections: pj[tok, f] = sum_d x[tok,d] Wall[f,d] ---
                pj = pj_ps.tile([P, NF], F32)
                for k in range(NBLK):
                    nc.tensor.matmul(pj, lhsT=xt_t[:, k, :], rhs=wt_s[:, k, :],
                                     start=(k == 0), stop=(k == NBLK - 1))

                # --- F = pj * s + C ---  (copy PSUM->SBUF first: the fused op
                # would need two HW waits, over the STT struct's limit)
                pjs = small.tile([P, NF], F32)
                nc.vector.tensor_copy(out=pjs, in_=pj)
                F = small.tile([P, NF], F32)
                nc.vector.scalar_tensor_tensor(out=F, in0=pjs, scalar=s[:, 0:1],
                                               in1=cv_s, op0=OP.mult, op1=OP.add)

                # --- sigmoids: [wr(4), dt_c, dt_d] = 1/(1+exp(-x)) ---
                E6 = small.tile([P, 6], F32)
                nc.scalar.activation(out=E6, in_=F[:, 32:38], func=AF.Exp, scale=-1.0)
                E6p = small.tile([P, 6], F32)
                nc.vector.tensor_scalar_add(E6p, E6, 1.0)
                SG = small.tile([P, 6], F32)
                nc.vector.reciprocal(out=SG, in_=E6p)
                # dt_c and -(dt_d/2) straight from SG, in parallel
                dtc = small.tile([P, 1], F32)
                nc.vector.tensor_scalar(out=dtc, in0=SG[:, 4:5],
                                        scalar1=DT_MAX - DT_MIN, scalar2=DT_MIN,
                                        op0=OP.mult, op1=OP.add)
                ndtd = small.tile([P, 1], F32)
                nc.vector.tensor_scalar(out=ndtd, in0=SG[:, 5:6],
                                        scalar1=-0.5 * (DT_MAX - DT_MIN),
                                        scalar2=-0.5 * DT_MIN,
                                        op0=OP.mult, op1=OP.add)

                # --- A1 = dt_c * (M - M^T) ---
                Fm = F[:, 0:16].rearrange("p (i j) -> p i j", i=4)
                FmT = F[:, 0:16].rearrange("p (i j) -> p j i", i=4)
                As = small.tile([P, 4, 4], F32)
                nc.vector.tensor_sub(As, Fm, FmT)
                A1 = small.tile([P, 4, 4], F32)
                nc.vector.tensor_scalar_mul(A1, As, dtc[:, 0:1])

                # --- K = R R^T on POOL, fully parallel to the sigmoid chain
                # (the dt_d scale is applied afterwards, off the K path) ---
                R3 = F[:, 16:32].rearrange("p (i j) -> p i j", i=4)
                KA = small.tile([P, 4, 4, 4], F32)  # [p, j, i, k]
                for j in range(4):
                    rij = R3[:, :, j:j + 1].broadcast_to((P, 4, 4))  # (i,k)->R[i,j]
                    rkj = R3[:, :, j:j + 1].transpose([0, 2, 1]).broadcast_to((P, 4, 4))  # (i,k)->R[k,j]
                    nc.gpsimd.tensor_mul(KA[:, j], rij, rkj)
                K01 = small.tile([P, 4, 4], F32)
                nc.gpsimd.tensor_add(K01, KA[:, 0], KA[:, 1])
                K23 = small.tile([P, 4, 4], F32)
                nc.gpsimd.tensor_add(K23, KA[:, 2], KA[:, 3])
                Kf = small.tile([P, 4, 4], F32)
                nc.gpsimd.tensor_add(Kf, K01, K23)

                # --- A = A1 + ndtd*K (fused) ---
                A = small.tile([P, 4, 4], F32)
                nc.vector.scalar_tensor_tensor(out=A, in0=Kf, scalar=ndtd[:, 0:1],
                                               in1=A1, op0=OP.mult, op1=OP.add)

                # --- u = wr + wr @ A;  D = A + ww (x) u ---
                wr = SG[:, 0:4]
                ww = F[:, 38:42]
                UB = small.tile([P, 4, 4], F32)  # [p, j, n]
                nc.vector.tensor_mul(
                    UB,
                    wr.unsqueeze(1).broadcast_to((P, 4, 4)),
                    A.rearrange("p n j -> p j n"),
                )
                u0 = small.tile([P, 4], F32)
                nc.vector.tensor_reduce(out=u0, in_=UB, axis=mybir.AxisListType.X,
                                        op=OP.add)
                u = small.tile([P, 4], F32)
                nc.vector.tensor_add(u, u0, wr)
                W16 = small.tile([P, 4, 4], F32)
                nc.gpsimd.tensor_mul(
                    W16,
                    ww.unsqueeze(2).broadcast_to((P, 4, 4)),
                    u.unsqueeze(1).broadcast_to((P, 4, 4)),
                )
                Dm = small.tile([P, 4, 4], F32)
                nc.vector.tensor_add(Dm, A, W16)

                # --- diag matrices: dg[p, i, j, c] = ident[p, c] * D[p, i, j] ---
                # split DVE/POOL so the build's latency (right before mixing)
                # is halved while only half the work lands on busy DVE
                dg = dgp.tile([P, 4, 4, P], BF16)
                for i in range(NSTR):
                    eng = nc.vector if i % 2 == 0 else nc.gpsimd
                    eng.tensor_mul(
                        dg[:, i],
                        id_s.unsqueeze(1).broadcast_to((P, 4, P)),
                        Dm[:, i].unsqueeze(2).broadcast_to((P, 4, P)),
                    )

                # --- mixing + residual add ---
                o_t = op_.tile([P, D], F32)
                for i in range(NSTR):
                    mx = mx_ps.tile([P, BS], F32, tag="mx")
                    for j in range(NSTR):
                        nc.tensor.matmul(mx, lhsT=dg[:, i, j, :],
                                         rhs=xh[:, j * BS:(j + 1) * BS],
                                         start=(j == 0), stop=(j == NSTR - 1))
                    sl = slice(i * BS, (i + 1) * BS)
                    if i == 1:
                        mb = mxs.tile([P, BS], F32, tag="mb")
                        nc.scalar.copy(out=mb, in_=mx)
                        nc.gpsimd.tensor_add(o_t[:, sl], mb, x_t[:, sl])
                    else:
                        nc.vector.tensor_add(o_t[:, sl], mx, x_t[:, sl])

                nc.sync.dma_start(out=out_d[t * P:(t + 1) * P, :], in_=o_t)
                _load(t + PF)

    nc.finalize()
    return nc


def prep_consts(inputs):
    """Pack the 42 projection rows + per-feature constants."""
    Wall = np.zeros((NF, D), np.float32)
    Wall[0:16] = np.asarray(inputs["W_conv"], np.float32)
    Wall[16:32] = np.asarray(inputs["W_diss"], np.float32)
    Wall[32:36] = float(np.asarray(inputs["alpha_read_in"])[0]) * np.asarray(
        inputs["W_read"], np.float32)
    Wall[36] = np.asarray(inputs["W_dt_c"], np.float32)[0]
    Wall[37] = np.asarray(inputs["W_dt_d"], np.float32)[0]
    Wall[38:42] = float(np.asarray(inputs["alpha_write_out"])[0]) * np.asarray(
        inputs["W_write"], np.float32)

    C = np.zeros((NF,), np.float32)
    C[0:16] = np.asarray(inputs["conserv_A"], np.float32)[0].reshape(16) + np.asarray(
        inputs["b_conv"], np.float32)
    C[16:32] = np.asarray(inputs["diss_A"], np.float32)[0].reshape(16) + np.asarray(
        inputs["b_diss"], np.float32)
    C[32:36] = np.asarray(inputs["read_in"], np.float32).reshape(4)
    C[36] = float(np.asarray(inputs["log_dt_c"])[0, 0]) + float(
        np.asarray(inputs["b_dt_c"])[0])
    C[37] = float(np.asarray(inputs["log_dt_d"])[0, 0]) + float(
        np.asarray(inputs["b_dt_d"])[0])
    C[38:42] = np.asarray(inputs["write_out"], np.float32).reshape(4)

    # wt[p, k, f] = Wall[f, k*128 + p], flattened to [128, 16*48]
    wt = np.ascontiguousarray(
        Wall.T.reshape(NBLK, P, NF).transpose(1, 0, 2).reshape(P, NBLK * NF)
    ).astype(ml_dtypes.bfloat16)
    cv = np.ascontiguousarray(np.broadcast_to(C[None, :], (P, NF))).astype(np.float32)
    ident = np.eye(P, dtype=ml_dtypes.bfloat16)
    return wt, cv, ident


def kernel(**inputs):
    global LAST_RESULTS
    x = np.asarray(inputs["x"], np.float32)
    B, T, _ = x.shape
    tok = B * T
    tpc = tok // NCORES
    xf = np.ascontiguousarray(x.reshape(tok, D))
    shards = xf.reshape(NCORES, tpc, D)

    wt, cv, ident = prep_consts(inputs)

    if tpc not in _NC_CACHE:
        _NC_CACHE[tpc] = build_nc(tpc)
    nc = _NC_CACHE[tpc]

    in_maps = [
        {"x": np.ascontiguousarray(shards[i]), "wt": wt, "cvec": cv, "ident": ident}
        for i in range(NCORES)
    ]
    res = run_bass_kernel_spmd(nc, in_maps, core_ids=list(range(NCORES)), trace=TRACE)
    LAST_RESULTS = res
    out = np.concatenate([r["out"] for r in res.results], axis=0)
    return out.reshape(B, T, D).astype(np.float32)
